# revision 4
# baseline (speedup 1.0000x reference)
"""AR(16) Gaussian log-likelihood kernel for Trainium2, 8 NeuronCores.

Math: out[b, t] = C - ((s[b,t] - sum_{k=1..16} phi_k s[b,t-k]) * invsc)^2
  with C = -0.5*log(2*pi*sigma^2), invsc = 1/(sqrt(2)*sigma).

Strategy (pure data parallel, 32 rows per core):
  - Each core's [32, 65536] shard is viewed as 128 streams of L=16384
    contiguous samples (4 segments per row); the host pre-casts to bf16
    (St[128, 16384]) and supplies a tiny [128, 32] halo block (previous
    stream's tail, zeros at row starts).
  - Input arrives via plain HWDGE DMAs on the SP ring in 8 chunks of
    [128, 2080] bf16 (2048 cols + 32-col halo overlap) - no SWDGE cast,
    half the HBM read traffic of the f32 baseline.
  - DVE stream-transposes each chunk (32x32 blocks): partition x of a
    32-group then holds samples 32C + x, giving the matmul a stride-1
    rhs with a 2-matrix banded-Toeplitz structure (dlt = 0 / -1).
  - TensorE: 2 accumulating block-diagonal [128,128] bf16 matmuls per
    512-col PSUM bank (kron(I4, T_dlt)), weight-grouped (4 banks per
    LDWEIGHTS target) so weight loads amortize.
  - ScalarE squares PSUM->SBUF (bf16); the C - x epilogue is split
    between DVE (tensor_scalar, 4x bf16) and ACT (Copy, scale=-1,
    bias=C) to balance engine load.
  - Output (bf16, still block-transposed) leaves on the gpsimd (SWDGE)
    ring so it never queues behind the input stream; the host
    de-interleaves with a pure reshape/transpose and upcasts to f32.
"""

import math

import numpy as np

import concourse.bass as bass
import concourse.tile as tile
from concourse import bacc, mybir
from concourse.bass_utils import run_bass_kernel_spmd

F32 = mybir.dt.float32
BF16 = mybir.dt.bfloat16

P = 16  # AR order
B_FULL, T_FULL = 256, 65536
N_CORES = 8
B_CORE = B_FULL // N_CORES   # 32 rows per core
SEG = 4                      # segments per row -> 128 streams per core
L = T_FULL // SEG            # 16384 samples per stream
NCOL = B_CORE * T_FULL // 128  # 16384 output cols per partition
NCH = 8                      # pipeline chunks
CH = NCOL // NCH             # 2048 output cols per chunk
CHW = CH + 32                # chunk cols incl. 32-col halo
EPI_PLAN = ["gps", "gps", "gps", "dve", "gps", "act", "dve", "act"]


def build_nc():
    nc = bacc.Bacc(
        "TRN2", target_bir_lowering=False, debug=False, enable_asserts=False
    )
    s_h = nc.declare_dram_parameter("s", [128, L + 32], BF16, isOutput=False)
    toep_h = nc.declare_dram_parameter("toep", [128, 256], BF16, isOutput=False)
    cvec_h = nc.declare_dram_parameter("cvec", [128, 1], F32, isOutput=False)
    out_h = nc.declare_dram_parameter("out", [128, NCOL], BF16, isOutput=True)

    from contextlib import ExitStack

    with tile.TileContext(nc) as tc, ExitStack() as ctx:
        const_pool = ctx.enter_context(tc.tile_pool(name="const", bufs=1))
        in_pool = ctx.enter_context(tc.tile_pool(name="inp", bufs=NCH))
        st_pool = ctx.enter_context(tc.tile_pool(name="stp", bufs=4))
        sq_pool = ctx.enter_context(tc.tile_pool(name="sqp", bufs=3))
        aff_pool = ctx.enter_context(tc.tile_pool(name="affp", bufs=3))
        psum_pool = ctx.enter_context(
            tc.tile_pool(name="psum", bufs=2, space="PSUM")
        )

        toep = const_pool.tile([128, 256], BF16)
        nc.sync.dma_start(out=toep[:, :], in_=toep_h.ap())
        cvec = const_pool.tile([128, 1], F32)
        nc.sync.dma_start(out=cvec[:, :], in_=cvec_h.ap())

        # all input DMAs issued up-front on the SP ring: deep queue, the
        # input stream never starves between chunks
        nats = []
        for k in range(NCH):
            nat = in_pool.tile([128, CHW], BF16, tag="nat", name=f"nat{k}")
            src_ap = bass.AP(s_h, CH * k, [[L + 32, 128], [1, CHW]])
            eng = nc.sync if k % 2 == 0 else nc.scalar
            eng.dma_start(out=nat[:, :], in_=src_ap)
            nats.append(nat)

        c_const = None  # filled below from cvec values host-side via bias imm

        affs = [None] * NCH
        sqs = [None] * NCH

        def emit_tail(k):
            """Epilogue + output DMA for chunk k (split across DVE/ACT)."""
            sq = sqs[k]
            aff = aff_pool.tile([128, CH], BF16, tag="aff")
            kind = EPI_PLAN[k]
            if kind == "act":
                # ACT epilogue: out = Copy(in * -1 + C)
                nc.scalar.activation(
                    aff[:, :],
                    sq[:, :],
                    mybir.ActivationFunctionType.Copy,
                    bias=float(_EPI_BIAS[0]),
                    scale=-1.0,
                )
            else:
                eng = nc.vector if kind == "dve" else nc.gpsimd
                eng.tensor_scalar(
                    aff[:, :],
                    sq[:, :],
                    -1.0,
                    cvec[:, :],
                    op0=mybir.AluOpType.mult,
                    op1=mybir.AluOpType.add,
                )
            out_view = bass.AP(out_h, k * CH, [[NCOL, 128], [1, CH]])
            nc.sync.dma_start(out=out_view, in_=aff[:, :])

        for k in range(NCH):
            nat = nats[k]
            st = st_pool.tile([128, CHW], BF16, tag="st")
            if k == 0:
                # split the head-of-pipeline transpose so compute starts
                # as soon as the first half of chunk 0 lands
                H2 = CHW // 2 - CHW // 2 % 32
                nc.vector.transpose(st[:, :H2], nat[:, :H2])
                nc.vector.transpose(st[:, H2:], nat[:, H2:])
            else:
                nc.vector.transpose(st[:, :], nat[:, :])

            q = psum_pool.tile([128, CH], F32, tag="q")
            # weight-grouped: 4 banks of W0 (start), then 4 of W1 (stop)
            for j in range(4):
                nc.tensor.matmul(
                    q[:, 512 * j : 512 * j + 512],
                    toep[:, 0:128],
                    st[:, 512 * j + 32 : 512 * j + 544],
                    start=True,
                    stop=False,
                    skip_group_check=True,
                )
            for j in range(4):
                nc.tensor.matmul(
                    q[:, 512 * j : 512 * j + 512],
                    toep[:, 128:256],
                    st[:, 512 * j : 512 * j + 512],
                    start=False,
                    stop=True,
                    skip_group_check=True,
                )
            sq = sq_pool.tile([128, CH], BF16, tag="sq")
            sqs[k] = sq
            nc.scalar.activation(
                sq[:, :], q[:, :], mybir.ActivationFunctionType.Square
            )
            if k >= 1:
                emit_tail(k - 1)
        emit_tail(NCH - 1)

    nc.compile()
    return nc


_EPI_BIAS = [0.0]  # C constant, set before build_nc() is called


def make_consts(coeffs: np.ndarray, noise_std: float):
    """Host-side O(1) prep: block-diagonal banded-Toeplitz filters."""
    import ml_dtypes

    coeffs = np.asarray(coeffs, dtype=np.float64).reshape(-1)
    p = coeffs.shape[0]
    sigma = float(noise_std)
    invsc = 1.0 / (math.sqrt(2.0) * sigma)
    c_const = -0.5 * math.log(2.0 * math.pi * sigma * sigma)
    h = np.zeros(p + 1, dtype=np.float64)
    h[0] = -invsc
    h[1:] = invsc * coeffs

    T0 = np.zeros((32, 32), dtype=np.float64)
    T1 = np.zeros((32, 32), dtype=np.float64)
    for k in range(32):
        for m in range(32):
            lag = m - k
            if 0 <= lag <= p:
                T0[k, m] = h[lag]
            lag2 = m - k + 32
            if 0 <= lag2 <= p:
                T1[k, m] = h[lag2]
    W0 = np.kron(np.eye(4), T0)
    W1 = np.kron(np.eye(4), T1)
    toep = np.concatenate([W0, W1], axis=1).astype(ml_dtypes.bfloat16)
    cvec = np.full((128, 1), c_const, dtype=np.float32)
    return toep, cvec, c_const


def make_streams(s_core: np.ndarray):
    """[32, 65536] f32 -> [128, 32+16384] bf16 (32-col halo prepended)."""
    import ml_dtypes

    St = np.ascontiguousarray(s_core).reshape(128, L).astype(ml_dtypes.bfloat16)
    pad = np.zeros((128, 32 + L), dtype=ml_dtypes.bfloat16)
    pad[:, 32:] = St
    idx = np.arange(128)
    sel = idx % SEG != 0
    pad[sel, :32] = St[idx[sel] - 1, -32:]
    return pad


def unshard_core(arr: np.ndarray) -> np.ndarray:
    """De-interleave one core's [128, 16384] block-transposed output back
    to [32, 65536]. Pure reshape/transpose."""
    A = arr.reshape(4, 32, L // 32, 32)                 # [a, m, C, y]
    O = np.ascontiguousarray(A.transpose(0, 3, 2, 1)).reshape(128, L)
    return O.reshape(B_CORE, T_FULL)


_NC_CACHE: dict = {}


def _get_nc(c_const):
    key = round(float(c_const), 9)
    if key not in _NC_CACHE:
        _EPI_BIAS[0] = float(c_const)
        _NC_CACHE[key] = build_nc()
    return _NC_CACHE[key]


def run_on_hw(s, coeffs, noise_std, trace=False, tmpdir=None):
    """Shard across 8 cores, run, gather. Returns (out, BassKernelResults)."""
    s = np.ascontiguousarray(np.asarray(s, dtype=np.float32))
    b_full, t_len = s.shape
    b_core = b_full // N_CORES
    toep, cvec, c_const = make_consts(coeffs, float(np.asarray(noise_std)))
    nc = _get_nc(c_const)
    in_maps = []
    for i in range(N_CORES):
        St = make_streams(s[i * b_core : (i + 1) * b_core])
        in_maps.append({"s": St, "toep": toep, "cvec": cvec})
    res = run_bass_kernel_spmd(
        nc, in_maps, core_ids=list(range(N_CORES)), trace=trace, tmpdir=tmpdir
    )
    out = np.concatenate(
        [
            unshard_core(np.asarray(res.results[i]["out"], dtype=np.float32))
            for i in range(N_CORES)
        ],
        axis=0,
    )
    return out, res


def kernel(s, coeffs, noise_std):
    out, _ = run_on_hw(s, coeffs, noise_std)
    return out


# revision 6
# speedup vs baseline: 1.0599x; 1.0599x over previous
"""AR(16) Gaussian log-likelihood kernel for Trainium2, 8 NeuronCores.

Math: out[b, t] = C - ((s[b,t] - sum_{k=1..16} phi_k s[b,t-k]) * invsc)^2
  with C = -0.5*log(2*pi*sigma^2), invsc = 1/(sqrt(2)*sigma).

Strategy (pure data parallel, 32 rows per core):
  - Each core's [32, 65536] shard is viewed as 128 streams of L=16384
    contiguous samples (4 segments per row); the host pre-casts to bf16
    (St[128, 16384]) and supplies a tiny [128, 32] halo block (previous
    stream's tail, zeros at row starts).
  - Input arrives via plain HWDGE DMAs on the SP ring in 8 chunks of
    [128, 2080] bf16 (2048 cols + 32-col halo overlap) - no SWDGE cast,
    half the HBM read traffic of the f32 baseline.
  - DVE stream-transposes each chunk (32x32 blocks): partition x of a
    32-group then holds samples 32C + x, giving the matmul a stride-1
    rhs with a 2-matrix banded-Toeplitz structure (dlt = 0 / -1).
  - TensorE: 2 accumulating block-diagonal [128,128] bf16 matmuls per
    512-col PSUM bank (kron(I4, T_dlt)), weight-grouped (4 banks per
    LDWEIGHTS target) so weight loads amortize.
  - ScalarE squares PSUM->SBUF (bf16); the C - x epilogue is split
    between DVE (tensor_scalar, 4x bf16) and ACT (Copy, scale=-1,
    bias=C) to balance engine load.
  - Output (bf16, still block-transposed) leaves on the gpsimd (SWDGE)
    ring so it never queues behind the input stream; the host
    de-interleaves with a pure reshape/transpose and upcasts to f32.
"""

import math

import numpy as np

import concourse.bass as bass
import concourse.tile as tile
from concourse import bacc, mybir
from concourse.bass_utils import run_bass_kernel_spmd

F32 = mybir.dt.float32
BF16 = mybir.dt.bfloat16

P = 16  # AR order
B_FULL, T_FULL = 256, 65536
N_CORES = 8
B_CORE = B_FULL // N_CORES   # 32 rows per core
SEG = 4                      # segments per row -> 128 streams per core
L = T_FULL // SEG            # 16384 samples per stream
NCOL = B_CORE * T_FULL // 128  # 16384 output cols per partition
# non-uniform chunks: small first (early pipeline start) and last (short tail)
CHUNKS = [1024] + [2048] * 7 + [1024]
NCH = len(CHUNKS)
OFFS = [sum(CHUNKS[:i]) for i in range(NCH)]
EPI_PLAN = ["dve", "gps", "gps", "gps", "gps", "gps", "gps", "gps", "dve"]
OUT_ENG = ["gps", "gps", "gps", "gps", "gps", "syn", "syn", "syn", "syn"]


def build_nc():
    nc = bacc.Bacc(
        "TRN2", target_bir_lowering=False, debug=False, enable_asserts=False
    )
    s_h = nc.declare_dram_parameter("s", [128, L + 32], BF16, isOutput=False)
    toep_h = nc.declare_dram_parameter("toep", [128, 256], BF16, isOutput=False)
    cvec_h = nc.declare_dram_parameter("cvec", [128, 1], F32, isOutput=False)
    out_h = nc.declare_dram_parameter("out", [128, NCOL], BF16, isOutput=True)

    from contextlib import ExitStack

    with tile.TileContext(nc) as tc, ExitStack() as ctx:
        const_pool = ctx.enter_context(tc.tile_pool(name="const", bufs=1))
        in_pool = ctx.enter_context(tc.tile_pool(name="inp", bufs=NCH))
        st_pool = ctx.enter_context(tc.tile_pool(name="stp", bufs=4))
        sq_pool = ctx.enter_context(tc.tile_pool(name="sqp", bufs=3))
        aff_pool = ctx.enter_context(tc.tile_pool(name="affp", bufs=3))
        psum_pool = ctx.enter_context(
            tc.tile_pool(name="psum", bufs=2, space="PSUM")
        )

        toep = const_pool.tile([128, 256], BF16)
        cvec = const_pool.tile([128, 1], F32)

        # all input DMAs issued up-front, in chunk order, on the SP ring
        # alone (the two HWDGE rings arbitrate with strict priority, so
        # sharing the input stream across rings starves chunk 0)
        nats = []
        for k in range(NCH):
            w = CHUNKS[k] + 32
            nat = in_pool.tile([128, w], BF16, tag="nat", name=f"nat{k}")
            src_ap = bass.AP(s_h, OFFS[k], [[L + 32, 128], [1, w]])
            nc.sync.dma_start(out=nat[:, :], in_=src_ap)
            nats.append(nat)
            if k == 0:
                nc.sync.dma_start(out=toep[:, :], in_=toep_h.ap())
                nc.sync.dma_start(out=cvec[:, :], in_=cvec_h.ap())

        sqs = [None] * NCH

        def emit_tail(k):
            """Epilogue + output DMA for chunk k (DVE or gpsimd)."""
            sq = sqs[k]
            w = CHUNKS[k]
            aff = aff_pool.tile([128, w], BF16, tag="aff")
            eng = nc.vector if EPI_PLAN[k] == "dve" else nc.gpsimd
            eng.tensor_scalar(
                aff[:, :],
                sq[:, :],
                -1.0,
                cvec[:, :],
                op0=mybir.AluOpType.mult,
                op1=mybir.AluOpType.add,
            )
            out_view = bass.AP(out_h, OFFS[k], [[NCOL, 128], [1, w]])
            deng = nc.sync if OUT_ENG[k] == "syn" else nc.gpsimd
            deng.dma_start(out=out_view, in_=aff[:, :])

        for k in range(NCH):
            nat = nats[k]
            w = CHUNKS[k]
            st = st_pool.tile([128, w + 32], BF16, tag="st")
            nc.vector.transpose(st[:, :], nat[:, :])

            q = psum_pool.tile([128, w], F32, tag="q")
            nb = w // 512
            # weight-grouped: all banks of W0 (start), then all of W1 (stop)
            for j in range(nb):
                nc.tensor.matmul(
                    q[:, 512 * j : 512 * j + 512],
                    toep[:, 0:128],
                    st[:, 512 * j + 32 : 512 * j + 544],
                    start=True,
                    stop=False,
                    skip_group_check=True,
                )
            for j in range(nb):
                nc.tensor.matmul(
                    q[:, 512 * j : 512 * j + 512],
                    toep[:, 128:256],
                    st[:, 512 * j : 512 * j + 512],
                    start=False,
                    stop=True,
                    skip_group_check=True,
                )
            sq = sq_pool.tile([128, w], BF16, tag="sq")
            sqs[k] = sq
            nc.scalar.activation(
                sq[:, :], q[:, :], mybir.ActivationFunctionType.Square
            )
            if k >= 1:
                emit_tail(k - 1)
        emit_tail(NCH - 1)

    nc.compile()
    return nc


_EPI_BIAS = [0.0]  # C constant, set before build_nc() is called


def make_consts(coeffs: np.ndarray, noise_std: float):
    """Host-side O(1) prep: block-diagonal banded-Toeplitz filters."""
    import ml_dtypes

    coeffs = np.asarray(coeffs, dtype=np.float64).reshape(-1)
    p = coeffs.shape[0]
    sigma = float(noise_std)
    invsc = 1.0 / (math.sqrt(2.0) * sigma)
    c_const = -0.5 * math.log(2.0 * math.pi * sigma * sigma)
    h = np.zeros(p + 1, dtype=np.float64)
    h[0] = -invsc
    h[1:] = invsc * coeffs

    T0 = np.zeros((32, 32), dtype=np.float64)
    T1 = np.zeros((32, 32), dtype=np.float64)
    for k in range(32):
        for m in range(32):
            lag = m - k
            if 0 <= lag <= p:
                T0[k, m] = h[lag]
            lag2 = m - k + 32
            if 0 <= lag2 <= p:
                T1[k, m] = h[lag2]
    W0 = np.kron(np.eye(4), T0)
    W1 = np.kron(np.eye(4), T1)
    toep = np.concatenate([W0, W1], axis=1).astype(ml_dtypes.bfloat16)
    cvec = np.full((128, 1), c_const, dtype=np.float32)
    return toep, cvec, c_const


def make_streams(s_core: np.ndarray):
    """[32, 65536] f32 -> [128, 32+16384] bf16 (32-col halo prepended)."""
    import ml_dtypes

    St = np.ascontiguousarray(s_core).reshape(128, L).astype(ml_dtypes.bfloat16)
    pad = np.zeros((128, 32 + L), dtype=ml_dtypes.bfloat16)
    pad[:, 32:] = St
    idx = np.arange(128)
    sel = idx % SEG != 0
    pad[sel, :32] = St[idx[sel] - 1, -32:]
    return pad


def unshard_core(arr: np.ndarray) -> np.ndarray:
    """De-interleave one core's [128, 16384] block-transposed output back
    to [32, 65536]. Pure reshape/transpose."""
    A = arr.reshape(4, 32, L // 32, 32)                 # [a, m, C, y]
    O = np.ascontiguousarray(A.transpose(0, 3, 2, 1)).reshape(128, L)
    return O.reshape(B_CORE, T_FULL)


_NC_CACHE: dict = {}


def _get_nc(c_const):
    key = round(float(c_const), 9)
    if key not in _NC_CACHE:
        _EPI_BIAS[0] = float(c_const)
        _NC_CACHE[key] = build_nc()
    return _NC_CACHE[key]


def run_on_hw(s, coeffs, noise_std, trace=False, tmpdir=None):
    """Shard across 8 cores, run, gather. Returns (out, BassKernelResults)."""
    s = np.ascontiguousarray(np.asarray(s, dtype=np.float32))
    b_full, t_len = s.shape
    b_core = b_full // N_CORES
    toep, cvec, c_const = make_consts(coeffs, float(np.asarray(noise_std)))
    nc = _get_nc(c_const)
    in_maps = []
    for i in range(N_CORES):
        St = make_streams(s[i * b_core : (i + 1) * b_core])
        in_maps.append({"s": St, "toep": toep, "cvec": cvec})
    res = run_bass_kernel_spmd(
        nc, in_maps, core_ids=list(range(N_CORES)), trace=trace, tmpdir=tmpdir
    )
    out = np.concatenate(
        [
            unshard_core(np.asarray(res.results[i]["out"], dtype=np.float32))
            for i in range(N_CORES)
        ],
        axis=0,
    )
    return out, res


def kernel(s, coeffs, noise_std):
    out, _ = run_on_hw(s, coeffs, noise_std)
    return out


# revision 7
# speedup vs baseline: 1.1688x; 1.1028x over previous
"""AR(16) Gaussian log-likelihood kernel for Trainium2, 8 NeuronCores.

Math: out[b, t] = C - ((s[b,t] - sum_{k=1..16} phi_k s[b,t-k]) * invsc)^2
  with C = -0.5*log(2*pi*sigma^2), invsc = 1/(sqrt(2)*sigma).

Strategy (pure data parallel, 32 rows per core):
  - Each core's [32, 65536] shard is viewed as 128 streams of L=16384
    contiguous samples (4 segments per row); the host pre-casts to bf16
    (St[128, 16384]) and supplies a tiny [128, 32] halo block (previous
    stream's tail, zeros at row starts).
  - Input arrives via plain HWDGE DMAs on the SP ring in 8 chunks of
    [128, 2080] bf16 (2048 cols + 32-col halo overlap) - no SWDGE cast,
    half the HBM read traffic of the f32 baseline.
  - DVE stream-transposes each chunk (32x32 blocks): partition x of a
    32-group then holds samples 32C + x, giving the matmul a stride-1
    rhs with a 2-matrix banded-Toeplitz structure (dlt = 0 / -1).
  - TensorE: 2 accumulating block-diagonal [128,128] bf16 matmuls per
    512-col PSUM bank (kron(I4, T_dlt)), weight-grouped (4 banks per
    LDWEIGHTS target) so weight loads amortize.
  - ScalarE squares PSUM->SBUF (bf16); the C - x epilogue is split
    between DVE (tensor_scalar, 4x bf16) and ACT (Copy, scale=-1,
    bias=C) to balance engine load.
  - Output (bf16, still block-transposed) leaves on the gpsimd (SWDGE)
    ring so it never queues behind the input stream; the host
    de-interleaves with a pure reshape/transpose and upcasts to f32.
"""

import math

import numpy as np

import concourse.bass as bass
import concourse.tile as tile
from concourse import bacc, mybir
from concourse.bass_utils import run_bass_kernel_spmd

F32 = mybir.dt.float32
BF16 = mybir.dt.bfloat16

P = 16  # AR order
B_FULL, T_FULL = 256, 65536
N_CORES = 8
B_CORE = B_FULL // N_CORES   # 32 rows per core
SEG = 4                      # segments per row -> 128 streams per core
L = T_FULL // SEG            # 16384 samples per stream
NCOL = B_CORE * T_FULL // 128  # 16384 output cols per partition
# non-uniform chunks: small first (early pipeline start) and last (short tail)
CHUNKS = [1024] + [2048] * 7 + [512, 512]
NCH = len(CHUNKS)
OFFS = [sum(CHUNKS[:i]) for i in range(NCH)]
EPI_PLAN = ["dve", "gps", "gps", "gps", "gps", "gps", "dve", "dve", "dve", "dve"]


def build_nc():
    nc = bacc.Bacc(
        "TRN2", target_bir_lowering=False, debug=False, enable_asserts=False
    )
    s_h = nc.declare_dram_parameter("s", [128, L + 32], BF16, isOutput=False)
    toep_h = nc.declare_dram_parameter("toep", [128, 256], BF16, isOutput=False)
    cvec_h = nc.declare_dram_parameter("cvec", [128, 1], F32, isOutput=False)
    out_h = nc.declare_dram_parameter("out", [128, NCOL], BF16, isOutput=True)

    from contextlib import ExitStack

    with tile.TileContext(nc) as tc, ExitStack() as ctx:
        const_pool = ctx.enter_context(tc.tile_pool(name="const", bufs=1))
        in_pool = ctx.enter_context(tc.tile_pool(name="inp", bufs=NCH))
        st_pool = ctx.enter_context(tc.tile_pool(name="stp", bufs=4))
        sq_pool = ctx.enter_context(tc.tile_pool(name="sqp", bufs=3))
        aff_pool = ctx.enter_context(tc.tile_pool(name="affp", bufs=3))
        psum_pool = ctx.enter_context(
            tc.tile_pool(name="psum", bufs=2, space="PSUM")
        )

        toep = const_pool.tile([128, 256], BF16)
        cvec = const_pool.tile([128, 1], F32)

        # all input DMAs issued up-front, in chunk order, on the SP ring
        # alone (the two HWDGE rings arbitrate with strict priority, so
        # sharing the input stream across rings starves chunk 0)
        nats = []
        for k in range(NCH):
            w = CHUNKS[k] + 32
            nat = in_pool.tile([128, w], BF16, tag="nat", name=f"nat{k}")
            src_ap = bass.AP(s_h, OFFS[k], [[L + 32, 128], [1, w]])
            nc.sync.dma_start(out=nat[:, :], in_=src_ap)
            nats.append(nat)
            if k == 1:
                nc.sync.dma_start(out=toep[:, :], in_=toep_h.ap())
                nc.sync.dma_start(out=cvec[:, :], in_=cvec_h.ap())

        sqs = [None] * NCH

        def emit_tail(k):
            """Epilogue + output DMA for chunk k (DVE or gpsimd)."""
            sq = sqs[k]
            w = CHUNKS[k]
            aff = aff_pool.tile([128, w], BF16, tag="aff")
            eng = nc.vector if EPI_PLAN[k] == "dve" else nc.gpsimd
            eng.tensor_scalar(
                aff[:, :],
                sq[:, :],
                -1.0,
                cvec[:, :],
                op0=mybir.AluOpType.mult,
                op1=mybir.AluOpType.add,
            )
            out_view = bass.AP(out_h, OFFS[k], [[NCOL, 128], [1, w]])
            nc.sync.dma_start(out=out_view, in_=aff[:, :])

        for k in range(NCH):
            nat = nats[k]
            w = CHUNKS[k]
            st = st_pool.tile([128, w + 32], BF16, tag="st")
            nc.vector.transpose(st[:, :], nat[:, :])

            q = psum_pool.tile([128, w], F32, tag="q")
            nb = w // 512
            # weight-grouped: all banks of W0 (start), then all of W1 (stop)
            for j in range(nb):
                nc.tensor.matmul(
                    q[:, 512 * j : 512 * j + 512],
                    toep[:, 0:128],
                    st[:, 512 * j + 32 : 512 * j + 544],
                    start=True,
                    stop=False,
                    skip_group_check=True,
                )
            for j in range(nb):
                nc.tensor.matmul(
                    q[:, 512 * j : 512 * j + 512],
                    toep[:, 128:256],
                    st[:, 512 * j : 512 * j + 512],
                    start=False,
                    stop=True,
                    skip_group_check=True,
                )
            sq = sq_pool.tile([128, w], BF16, tag="sq")
            sqs[k] = sq
            nc.scalar.activation(
                sq[:, :], q[:, :], mybir.ActivationFunctionType.Square
            )
            if k >= 1:
                emit_tail(k - 1)
        emit_tail(NCH - 1)

    nc.compile()
    return nc


_EPI_BIAS = [0.0]  # C constant, set before build_nc() is called


def make_consts(coeffs: np.ndarray, noise_std: float):
    """Host-side O(1) prep: block-diagonal banded-Toeplitz filters."""
    import ml_dtypes

    coeffs = np.asarray(coeffs, dtype=np.float64).reshape(-1)
    p = coeffs.shape[0]
    sigma = float(noise_std)
    invsc = 1.0 / (math.sqrt(2.0) * sigma)
    c_const = -0.5 * math.log(2.0 * math.pi * sigma * sigma)
    h = np.zeros(p + 1, dtype=np.float64)
    h[0] = -invsc
    h[1:] = invsc * coeffs

    T0 = np.zeros((32, 32), dtype=np.float64)
    T1 = np.zeros((32, 32), dtype=np.float64)
    for k in range(32):
        for m in range(32):
            lag = m - k
            if 0 <= lag <= p:
                T0[k, m] = h[lag]
            lag2 = m - k + 32
            if 0 <= lag2 <= p:
                T1[k, m] = h[lag2]
    W0 = np.kron(np.eye(4), T0)
    W1 = np.kron(np.eye(4), T1)
    toep = np.concatenate([W0, W1], axis=1).astype(ml_dtypes.bfloat16)
    cvec = np.full((128, 1), c_const, dtype=np.float32)
    return toep, cvec, c_const


def make_streams(s_core: np.ndarray):
    """[32, 65536] f32 -> [128, 32+16384] bf16 (32-col halo prepended)."""
    import ml_dtypes

    St = np.ascontiguousarray(s_core).reshape(128, L).astype(ml_dtypes.bfloat16)
    pad = np.zeros((128, 32 + L), dtype=ml_dtypes.bfloat16)
    pad[:, 32:] = St
    idx = np.arange(128)
    sel = idx % SEG != 0
    pad[sel, :32] = St[idx[sel] - 1, -32:]
    return pad


def unshard_core(arr: np.ndarray) -> np.ndarray:
    """De-interleave one core's [128, 16384] block-transposed output back
    to [32, 65536]. Pure reshape/transpose."""
    A = arr.reshape(4, 32, L // 32, 32)                 # [a, m, C, y]
    O = np.ascontiguousarray(A.transpose(0, 3, 2, 1)).reshape(128, L)
    return O.reshape(B_CORE, T_FULL)


_NC_CACHE: dict = {}


def _get_nc(c_const):
    key = round(float(c_const), 9)
    if key not in _NC_CACHE:
        _EPI_BIAS[0] = float(c_const)
        _NC_CACHE[key] = build_nc()
    return _NC_CACHE[key]


def run_on_hw(s, coeffs, noise_std, trace=False, tmpdir=None):
    """Shard across 8 cores, run, gather. Returns (out, BassKernelResults)."""
    s = np.ascontiguousarray(np.asarray(s, dtype=np.float32))
    b_full, t_len = s.shape
    b_core = b_full // N_CORES
    toep, cvec, c_const = make_consts(coeffs, float(np.asarray(noise_std)))
    nc = _get_nc(c_const)
    in_maps = []
    for i in range(N_CORES):
        St = make_streams(s[i * b_core : (i + 1) * b_core])
        in_maps.append({"s": St, "toep": toep, "cvec": cvec})
    res = run_bass_kernel_spmd(
        nc, in_maps, core_ids=list(range(N_CORES)), trace=trace, tmpdir=tmpdir
    )
    out = np.concatenate(
        [
            unshard_core(np.asarray(res.results[i]["out"], dtype=np.float32))
            for i in range(N_CORES)
        ],
        axis=0,
    )
    return out, res


def kernel(s, coeffs, noise_std):
    out, _ = run_on_hw(s, coeffs, noise_std)
    return out


# revision 9
# speedup vs baseline: 1.2129x; 1.0377x over previous
"""AR(16) Gaussian log-likelihood kernel for Trainium2, 8 NeuronCores.

Math: out[b, t] = C - ((s[b,t] - sum_{k=1..16} phi_k s[b,t-k]) * invsc)^2
  with C = -0.5*log(2*pi*sigma^2), invsc = 1/(sqrt(2)*sigma).

Strategy (pure data parallel, 32 rows per core):
  - Each core's [32, 65536] shard is viewed as 128 streams of L=16384
    contiguous samples (4 segments per row); the host pre-casts to bf16
    (St[128, 16384]) and supplies a tiny [128, 32] halo block (previous
    stream's tail, zeros at row starts).
  - Input arrives via plain HWDGE DMAs on the SP ring in 8 chunks of
    [128, 2080] bf16 (2048 cols + 32-col halo overlap) - no SWDGE cast,
    half the HBM read traffic of the f32 baseline.
  - DVE stream-transposes each chunk (32x32 blocks): partition x of a
    32-group then holds samples 32C + x, giving the matmul a stride-1
    rhs with a 2-matrix banded-Toeplitz structure (dlt = 0 / -1).
  - TensorE: 2 accumulating block-diagonal [128,128] bf16 matmuls per
    512-col PSUM bank (kron(I4, T_dlt)), weight-grouped (4 banks per
    LDWEIGHTS target) so weight loads amortize.
  - ScalarE squares PSUM->SBUF (bf16); the C - x epilogue is split
    between DVE (tensor_scalar, 4x bf16) and ACT (Copy, scale=-1,
    bias=C) to balance engine load.
  - Output (bf16, still block-transposed) leaves on the gpsimd (SWDGE)
    ring so it never queues behind the input stream; the host
    de-interleaves with a pure reshape/transpose and upcasts to f32.
"""

import math

import numpy as np

import concourse.bass as bass
import concourse.tile as tile
from concourse import bacc, mybir
from concourse.bass_utils import run_bass_kernel_spmd

F32 = mybir.dt.float32
BF16 = mybir.dt.bfloat16

P = 16  # AR order
B_FULL, T_FULL = 256, 65536
N_CORES = 8
B_CORE = B_FULL // N_CORES   # 32 rows per core
SEG = 4                      # segments per row -> 128 streams per core
L = T_FULL // SEG            # 16384 samples per stream
NCOL = B_CORE * T_FULL // 128  # 16384 output cols per partition
# non-uniform chunks: small first (early pipeline start) and last (short tail)
CHUNKS = [1024, 1024] + [2048] * 6 + [1024, 512, 512]
NCH = len(CHUNKS)
OFFS = [sum(CHUNKS[:i]) for i in range(NCH)]
EPI_PLAN = ["dve", "gps", "gps", "gps", "gps", "gps", "gps", "dve", "dve", "dve", "dve"]
SQ_ENG = ["act"] * NCH


def build_nc():
    nc = bacc.Bacc(
        "TRN2", target_bir_lowering=False, debug=False, enable_asserts=False
    )
    s_h = nc.declare_dram_parameter("s", [128, L + 32], BF16, isOutput=False)
    toep_h = nc.declare_dram_parameter("toep", [128, 256], BF16, isOutput=False)
    cvec_h = nc.declare_dram_parameter("cvec", [128, 1], F32, isOutput=False)
    out_h = nc.declare_dram_parameter("out", [128, NCOL], BF16, isOutput=True)

    from contextlib import ExitStack

    with tile.TileContext(nc) as tc, ExitStack() as ctx:
        const_pool = ctx.enter_context(tc.tile_pool(name="const", bufs=1))
        in_pool = ctx.enter_context(tc.tile_pool(name="inp", bufs=NCH))
        st_pool = ctx.enter_context(tc.tile_pool(name="stp", bufs=4))
        sq_pool = ctx.enter_context(tc.tile_pool(name="sqp", bufs=5))
        aff_pool = ctx.enter_context(tc.tile_pool(name="affp", bufs=6))
        psum_pool = ctx.enter_context(
            tc.tile_pool(name="psum", bufs=2, space="PSUM")
        )

        toep = const_pool.tile([128, 256], BF16)
        cvec = const_pool.tile([128, 1], F32)

        # all input DMAs issued up-front, in chunk order, on the SP ring
        # alone (the two HWDGE rings arbitrate with strict priority, so
        # sharing the input stream across rings starves chunk 0)
        nats = []
        for k in range(NCH):
            w = CHUNKS[k] + 32
            nat = in_pool.tile([128, w], BF16, tag="nat", name=f"nat{k}")
            src_ap = bass.AP(s_h, OFFS[k], [[L + 32, 128], [1, w]])
            nc.sync.dma_start(out=nat[:, :], in_=src_ap)
            nats.append(nat)
            if k == 1:
                nc.sync.dma_start(out=toep[:, :], in_=toep_h.ap())
                nc.sync.dma_start(out=cvec[:, :], in_=cvec_h.ap())

        sqs = [None] * NCH

        def emit_tail(k):
            """Epilogue + output DMA for chunk k (DVE or gpsimd)."""
            sq = sqs[k]
            w = CHUNKS[k]
            aff = aff_pool.tile([128, w], BF16, tag="aff")
            eng = nc.vector if EPI_PLAN[k] == "dve" else nc.gpsimd
            eng.tensor_scalar(
                aff[:, :],
                sq[:, :],
                -1.0,
                cvec[:, :],
                op0=mybir.AluOpType.mult,
                op1=mybir.AluOpType.add,
            )
            out_view = bass.AP(out_h, OFFS[k], [[NCOL, 128], [1, w]])
            nc.sync.dma_start(out=out_view, in_=aff[:, :])

        for k in range(NCH):
            nat = nats[k]
            w = CHUNKS[k]
            st = st_pool.tile([128, w + 32], BF16, tag="st")
            nc.vector.transpose(st[:, :], nat[:, :])

            q = psum_pool.tile([128, w], F32, tag="q")
            nb = w // 512
            # weight-grouped: all banks of W0 (start), then all of W1 (stop)
            for j in range(nb):
                nc.tensor.matmul(
                    q[:, 512 * j : 512 * j + 512],
                    toep[:, 0:128],
                    st[:, 512 * j + 32 : 512 * j + 544],
                    start=True,
                    stop=False,
                    skip_group_check=True,
                )
            for j in range(nb):
                nc.tensor.matmul(
                    q[:, 512 * j : 512 * j + 512],
                    toep[:, 128:256],
                    st[:, 512 * j : 512 * j + 512],
                    start=False,
                    stop=True,
                    skip_group_check=True,
                )
            sq = sq_pool.tile([128, w], BF16, tag="sq")
            sqs[k] = sq
            if SQ_ENG[k] == "dve":
                nc.vector.tensor_mul(sq[:, :], q[:, :], q[:, :])
            else:
                nc.scalar.activation(
                    sq[:, :], q[:, :], mybir.ActivationFunctionType.Square
                )
            if k >= 1:
                emit_tail(k - 1)
        emit_tail(NCH - 1)

    nc.compile()
    return nc


_EPI_BIAS = [0.0]  # C constant, set before build_nc() is called


def make_consts(coeffs: np.ndarray, noise_std: float):
    """Host-side O(1) prep: block-diagonal banded-Toeplitz filters."""
    import ml_dtypes

    coeffs = np.asarray(coeffs, dtype=np.float64).reshape(-1)
    p = coeffs.shape[0]
    sigma = float(noise_std)
    invsc = 1.0 / (math.sqrt(2.0) * sigma)
    c_const = -0.5 * math.log(2.0 * math.pi * sigma * sigma)
    h = np.zeros(p + 1, dtype=np.float64)
    h[0] = -invsc
    h[1:] = invsc * coeffs

    T0 = np.zeros((32, 32), dtype=np.float64)
    T1 = np.zeros((32, 32), dtype=np.float64)
    for k in range(32):
        for m in range(32):
            lag = m - k
            if 0 <= lag <= p:
                T0[k, m] = h[lag]
            lag2 = m - k + 32
            if 0 <= lag2 <= p:
                T1[k, m] = h[lag2]
    W0 = np.kron(np.eye(4), T0)
    W1 = np.kron(np.eye(4), T1)
    toep = np.concatenate([W0, W1], axis=1).astype(ml_dtypes.bfloat16)
    cvec = np.full((128, 1), c_const, dtype=np.float32)
    return toep, cvec, c_const


def make_streams(s_core: np.ndarray):
    """[32, 65536] f32 -> [128, 32+16384] bf16 (32-col halo prepended)."""
    import ml_dtypes

    St = np.ascontiguousarray(s_core).reshape(128, L).astype(ml_dtypes.bfloat16)
    pad = np.zeros((128, 32 + L), dtype=ml_dtypes.bfloat16)
    pad[:, 32:] = St
    idx = np.arange(128)
    sel = idx % SEG != 0
    pad[sel, :32] = St[idx[sel] - 1, -32:]
    return pad


def unshard_core(arr: np.ndarray) -> np.ndarray:
    """De-interleave one core's [128, 16384] block-transposed output back
    to [32, 65536]. Pure reshape/transpose."""
    A = arr.reshape(4, 32, L // 32, 32)                 # [a, m, C, y]
    O = np.ascontiguousarray(A.transpose(0, 3, 2, 1)).reshape(128, L)
    return O.reshape(B_CORE, T_FULL)


_NC_CACHE: dict = {}


def _get_nc(c_const):
    key = round(float(c_const), 9)
    if key not in _NC_CACHE:
        _EPI_BIAS[0] = float(c_const)
        _NC_CACHE[key] = build_nc()
    return _NC_CACHE[key]


def run_on_hw(s, coeffs, noise_std, trace=False, tmpdir=None):
    """Shard across 8 cores, run, gather. Returns (out, BassKernelResults)."""
    s = np.ascontiguousarray(np.asarray(s, dtype=np.float32))
    b_full, t_len = s.shape
    b_core = b_full // N_CORES
    toep, cvec, c_const = make_consts(coeffs, float(np.asarray(noise_std)))
    nc = _get_nc(c_const)
    in_maps = []
    for i in range(N_CORES):
        St = make_streams(s[i * b_core : (i + 1) * b_core])
        in_maps.append({"s": St, "toep": toep, "cvec": cvec})
    res = run_bass_kernel_spmd(
        nc, in_maps, core_ids=list(range(N_CORES)), trace=trace, tmpdir=tmpdir
    )
    out = np.concatenate(
        [
            unshard_core(np.asarray(res.results[i]["out"], dtype=np.float32))
            for i in range(N_CORES)
        ],
        axis=0,
    )
    return out, res


def kernel(s, coeffs, noise_std):
    out, _ = run_on_hw(s, coeffs, noise_std)
    return out
